# revision 29
# baseline (speedup 1.0000x reference)
"""Trainium2 Bass kernel for nn_KANLayer (B=16384, D=1024, K=8).

Math: the per-feature basis chain collapses algebraically:
    nl[b,i] = sum_k (x[b,i]*W1[i,k] + b1[i,k]) * W2[i,k]
            = x[b,i] * a[i] + c[i],   a = sum_k W1*W2, c = sum_k b1*W2
so the whole layer is ONE dense matmul with a fused diagonal + bias:
    out = x @ (lin_W.T + diag(a)) + (lin_b + c)

Precision strategy (validated numerically on the fixed seed-0 inputs,
rel err ~1e-2 vs the 2e-2 gate): split W_eff = W_off + diag(d).
  - x @ W_off runs on the PE in fp8 e4m3 with perf_mode=DoubleRow
    (2 fp8/cell, K=256 per matmul) — ~216ns per [256x128x512] matmul.
  - the diagonal term d[i]*x[b,i] is large (|d|~1 vs |W_off|~1/32), so
    it is applied at eviction from an fp16 copy of x with one DVE
    scalar_tensor_tensor op per tile.
  - output stored fp16 (|out|<=21, rel rounding err ~2^-11).

Layout: everything transposed — W is the PE-stationary operand, x^T the
moving one, so psum holds out^T[i, b] and the diagonal/bias become
per-partition scalars. Host transposes the output back.

DMA scheduling: each dma_start lands on one HW queue and all enqueued
transfers fair-share HBM bandwidth, so loads are split into pieces and
issued in priority order: tiny 64KB partition-split pieces for the
very first matmul's operands, then the rest of x8/w8, then fp16 x for
k-pair 0; fp16 x for k-pairs 1-3 is released from the ACT instruction
stream after earlier evictions. Stores ride the gpsimd ring.

Sharding: data-parallel over batch across 8 NeuronCores (2048 rows
each); weights replicated. No collectives.
"""

from contextlib import ExitStack

import numpy as np
import ml_dtypes

import concourse.bass as bass
import concourse.tile as tile
from concourse import bacc, mybir
from concourse.bass_utils import run_bass_kernel_spmd

B, D = 16384, 1024
NCORES = 8
BS = B // NCORES   # 2048 batch rows per core
P = 128
TP = 4             # contraction k-pairs (each pair = 256 rows via DoubleRow)
IB = D // P        # 8 output-feature blocks of 128
NBC = BS // 512    # 4 batch chunks of 512

FP8 = mybir.dt.float8e4
FP8_NP = ml_dtypes.float8_e4m3fn  # matches TRN fp8e4 within +-240
F16 = mybir.dt.float16

_CACHE = {}


def _build_nc():
    nc = bacc.Bacc("TRN2", target_bir_lowering=False, debug=False,
                   num_devices=NCORES)
    # x8: x^T per core in fp8 (PE moving operand), layout [p, t, 2, b] with
    # contraction index j = (2*t + sub)*128 + p; a per-(t) transfer is 4KB
    # per partition (big DMA lines = full bandwidth).
    x8 = nc.dram_tensor("x8", [P, TP, 2, BS], FP8,
                        kind="ExternalInput").ap()
    # xf: x^T in fp16 for the diagonal correction, same layout.
    xf = nc.dram_tensor("xf", [P, TP, 2, BS], F16,
                        kind="ExternalInput").ap()
    # W_off (diag zeroed), layout [p, ib, t, 2, i128]: one piece per ib
    # column block holding all k-pairs.
    w8 = nc.dram_tensor("w8", [P, IB, TP, 2, P], FP8,
                        kind="ExternalInput").ap()
    # diag + bias as per-partition columns packed in one tensor:
    # vb[p, 0:8] = d[ib*128+p], vb[p, 8:16] = bias[ib*128+p]
    vb = nc.dram_tensor("vb", [P, 2 * IB], mybir.dt.float32,
                        kind="ExternalInput").ap()
    # out^T fp16: out[p, ib, b] = result[b, ib*128+p]
    out = nc.dram_tensor("out", [P, IB, BS], F16, kind="ExternalOutput").ap()

    Act = mybir.ActivationFunctionType
    Alu = mybir.AluOpType
    DR = mybir.MatmulPerfMode.DoubleRow

    with tile.TileContext(nc) as tc, ExitStack() as ctx:
        cpool = ctx.enter_context(tc.tile_pool(name="cpool", bufs=1))
        opool = ctx.enter_context(tc.tile_pool(name="opool", bufs=3))
        tpool = ctx.enter_context(tc.tile_pool(name="tpool", bufs=8))
        ppool = ctx.enter_context(tc.tile_pool(name="ppool", bufs=8,
                                               space="PSUM"))

        x_t = [cpool.tile([P, 2, BS], FP8, tag=f"x{t}", name=f"x_t{t}")
               for t in range(TP)]
        w_t = cpool.tile([P, IB, TP, 2, P], FP8, tag="w", name="w_t")
        xf_t = [cpool.tile([P, 2, BS], F16, tag=f"xf{t}", name=f"xf_t{t}")
                for t in range(TP)]
        vb_t = cpool.tile([P, 2 * IB], mybir.dt.float32, tag="vb",
                          name="vb_t")
        dv_t = vb_t[:, :IB]
        bv_t = vb_t[:, IB:]

        # DMA model (measured): a single transfer with >=4KB-per-partition
        # lines reaches near-full bandwidth (~512KB in ~1.5-3us); small or
        # thin-line pieces pay large fixed costs, each ring allows only ~4
        # in-flight transfers, and each trigger costs ~0.6-0.8us of
        # sequencer time. So: few, fat transfers, issued in the order the
        # data is consumed, balanced across the sync/scalar rings (loads)
        # with stores on the otherwise-idle gpsimd ring.
        #
        # gpsimd ring: the tiny thin-line diag/bias vector (it would poison
        # the head of a load ring), then stores later.
        nc.gpsimd.dma_start(out=vb_t, in_=vb)
        # sync ring: x8 per k-pair (512KB, 4KB/part), then fp16 x for
        # k-pairs 2-3 per (t,sub) = one eviction block each (512KB).
        for t in range(TP):
            nc.sync.dma_start(out=x_t[t], in_=x8[:, t])
        for t in (2, 3):
            for sub in range(2):
                nc.sync.dma_start(out=xf_t[t][:, sub], in_=xf[:, t, sub])
        # scalar ring: w8 per ib-pair (256KB, 2KB/part), then fp16 x for
        # k-pairs 0-1.
        for i2 in range(IB // 2):
            nc.scalar.dma_start(out=w_t[:, bass.ts(i2, 2)],
                                in_=w8[:, bass.ts(i2, 2)])
        for t in (0, 1):
            for sub in range(2):
                nc.scalar.dma_start(out=xf_t[t][:, sub], in_=xf[:, t, sub])

        # PE pre-warm with fp8 DoubleRow matmuls (tile zeroed on the idle
        # gpsimd engine so warmup starts as soon as the PE boots).
        warm = cpool.tile([P, 2, 512], FP8, tag="warm", name="warm")
        nc.gpsimd.memset(warm, 0.0)
        warm_ps = ppool.tile([P, 512], mybir.dt.float32, tag="ps",
                             name="warm_ps")
        NWARM = 5
        for i in range(NWARM):
            nc.tensor.matmul(warm_ps, lhsT=warm[:, :, :P], rhs=warm,
                             start=(i == 0), stop=(i == NWARM - 1),
                             perf_mode=DR)

        for ib in range(IB):
            psums = [ppool.tile([P, 512], mybir.dt.float32, tag="ps",
                                name=f"ps{ib}_{bc}") for bc in range(NBC)]
            for t in range(TP):
                for bc in range(NBC):
                    nc.tensor.matmul(
                        psums[bc],
                        lhsT=w_t[:, ib, t],
                        rhs=x_t[t][:, :, bass.ts(bc, 512)],
                        start=(t == 0),
                        stop=(t == TP - 1),
                        perf_mode=DR,
                    )
            # eviction: out^T[i,b] = psum + d[i]*x_f16[i,b] + bias[i]
            t8, sub = divmod(ib, 2)
            o_t = opool.tile([P, BS], F16, tag="o", name=f"o_t{ib}")
            for bc in range(NBC):
                tb = tpool.tile([P, 512], F16, tag="tb",
                                name=f"tb{ib}_{bc}")
                nc.scalar.activation(tb, psums[bc], Act.Identity,
                                     bias=bv_t[:, ib:ib + 1], scale=1.0)
                nc.vector.scalar_tensor_tensor(
                    o_t[:, bass.ts(bc, 512)],
                    in0=xf_t[t8][:, sub, bass.ts(bc, 512)],
                    scalar=dv_t[:, ib:ib + 1],
                    in1=tb, op0=Alu.mult, op1=Alu.add)
                if ib == IB - 1:
                    # kernel tail: store per 512-chunk as soon as each
                    # chunk's eviction lands
                    nc.gpsimd.dma_start(out=out[:, ib, bass.ts(bc, 512)],
                                        in_=o_t[:, bass.ts(bc, 512)])
            if ib < IB - 1:
                nc.gpsimd.dma_start(out=out[:, ib], in_=o_t)

    nc.compile()
    return nc


def _get_nc():
    if "nc" not in _CACHE:
        _CACHE["nc"] = _build_nc()
    return _CACHE["nc"]


def _prep_inputs(x, lin_W, lin_b, W1, b1, W2):
    """Host-side prep: fold the basis chain, split W into off-diag + diag,
    quantize to fp8/fp16, and lay out transposed per core."""
    x = np.asarray(x, dtype=np.float32)
    lin_W = np.asarray(lin_W, dtype=np.float32)
    a = np.sum(np.asarray(W1, np.float32) * np.asarray(W2, np.float32),
               axis=1)
    c = np.sum(np.asarray(b1, np.float32) * np.asarray(W2, np.float32),
               axis=1)
    W_eff = np.ascontiguousarray(lin_W.T)
    idx = np.arange(D)
    W_eff[idx, idx] += a
    d = W_eff[idx, idx].copy()
    W_off = W_eff
    W_off[idx, idx] = 0.0
    bias = (np.asarray(lin_b, np.float32) + c).astype(np.float32)

    x8 = x.astype(FP8_NP)
    xf = x.astype(np.float16)
    w8 = W_off.astype(FP8_NP)

    # w8 dram layout [p, ib, t, 2, i128]: j = (2*t+sub)*128 + p,
    # i = ib*128 + i128
    w8_dev = np.ascontiguousarray(
        w8.reshape(TP, 2, P, IB, P).transpose(2, 3, 0, 1, 4))
    vb_dev = np.ascontiguousarray(
        np.concatenate([d.reshape(IB, P).T, bias.reshape(IB, P).T], axis=1))

    def xpose(arr):  # [NCORES*BS, D] -> per-core [p, t, 2, b]
        t = arr.reshape(NCORES, BS, TP, 2, P)
        return np.ascontiguousarray(t.transpose(0, 4, 2, 3, 1))

    x8_dev = xpose(x8)
    xf_dev = xpose(xf)

    return [
        {"x8": x8_dev[i], "xf": xf_dev[i], "w8": w8_dev, "vb": vb_dev}
        for i in range(NCORES)
    ]


def kernel(x, lin_W, lin_b, W1, b1, W2):
    in_maps = _prep_inputs(x, lin_W, lin_b, W1, b1, W2)
    nc = _get_nc()
    res = run_bass_kernel_spmd(nc, in_maps, core_ids=list(range(NCORES)))
    # out^T [p, ib, b] per core -> [b_global, ib*128+p]
    o = np.stack([r["out"] for r in res.results])  # [cores, P, IB, BS] fp16
    o = o.astype(np.float32).transpose(0, 3, 2, 1).reshape(B, D)
    return np.ascontiguousarray(o)


# revision 31
# speedup vs baseline: 1.0960x; 1.0960x over previous
"""Trainium2 Bass kernel for nn_KANLayer (B=16384, D=1024, K=8).

Math: the per-feature basis chain collapses algebraically:
    nl[b,i] = sum_k (x[b,i]*W1[i,k] + b1[i,k]) * W2[i,k]
            = x[b,i] * a[i] + c[i],   a = sum_k W1*W2, c = sum_k b1*W2
so the whole layer is ONE dense matmul with a fused diagonal + bias:
    out = x @ (lin_W.T + diag(a)) + (lin_b + c)

Precision strategy (validated numerically on the fixed seed-0 inputs,
rel err ~1e-2 vs the 2e-2 gate): split W_eff = W_off + diag(d).
  - x @ W_off runs on the PE in fp8 e4m3 with perf_mode=DoubleRow
    (2 fp8/cell, K=256 per matmul) — ~216ns per [256x128x512] matmul.
  - the diagonal term d[i]*x[b,i] is large (|d|~1 vs |W_off|~1/32), so
    it is applied at eviction from an fp16 copy of x with one DVE
    scalar_tensor_tensor op per tile.
  - output stored fp16 (|out|<=21, rel rounding err ~2^-11).

Layout: everything transposed — W is the PE-stationary operand, x^T the
moving one, so psum holds out^T[i, b] and the diagonal/bias become
per-partition scalars. Host transposes the output back.

DMA scheduling: each dma_start lands on one HW queue and all enqueued
transfers fair-share HBM bandwidth, so loads are split into pieces and
issued in priority order: tiny 64KB partition-split pieces for the
very first matmul's operands, then the rest of x8/w8, then fp16 x for
k-pair 0; fp16 x for k-pairs 1-3 is released from the ACT instruction
stream after earlier evictions. Stores ride the gpsimd ring.

Sharding: data-parallel over batch across 8 NeuronCores (2048 rows
each); weights replicated. No collectives.
"""

from contextlib import ExitStack

import numpy as np
import ml_dtypes

import concourse.bass as bass
import concourse.tile as tile
from concourse import bacc, mybir
from concourse.bass_utils import run_bass_kernel_spmd

B, D = 16384, 1024
NCORES = 8
BS = B // NCORES   # 2048 batch rows per core
P = 128
TP = 4             # contraction k-pairs (each pair = 256 rows via DoubleRow)
IB = D // P        # 8 output-feature blocks of 128
NBC = BS // 512    # 4 batch chunks of 512

FP8 = mybir.dt.float8e4
FP8_NP = ml_dtypes.float8_e4m3fn  # matches TRN fp8e4 within +-240
F16 = mybir.dt.float16

_CACHE = {}


def _build_nc():
    nc = bacc.Bacc("TRN2", target_bir_lowering=False, debug=False,
                   num_devices=NCORES)
    # x8: x^T per core in fp8 (PE moving operand), layout [p, t, 2, b] with
    # contraction index j = (2*t + sub)*128 + p; a per-(t) transfer is 4KB
    # per partition (big DMA lines = full bandwidth).
    x8 = nc.dram_tensor("x8", [P, TP, 2, BS], FP8,
                        kind="ExternalInput").ap()
    # xf: x^T in fp16 for the diagonal correction, same layout.
    xf = nc.dram_tensor("xf", [P, TP, 2, BS], F16,
                        kind="ExternalInput").ap()
    # W_off (diag zeroed), layout [p, ib, t, 2, i128]: one piece per ib
    # column block holding all k-pairs.
    w8 = nc.dram_tensor("w8", [P, IB, TP, 2, P], FP8,
                        kind="ExternalInput").ap()
    # diag + bias as per-partition columns packed in one tensor:
    # vb[p, 0:8] = d[ib*128+p], vb[p, 8:16] = bias[ib*128+p]
    vb = nc.dram_tensor("vb", [P, 2 * IB], mybir.dt.float32,
                        kind="ExternalInput").ap()
    # out^T fp16: out[p, ib, b] = result[b, ib*128+p]
    out = nc.dram_tensor("out", [P, IB, BS], F16, kind="ExternalOutput").ap()

    Act = mybir.ActivationFunctionType
    Alu = mybir.AluOpType
    DR = mybir.MatmulPerfMode.DoubleRow

    with tile.TileContext(nc) as tc, ExitStack() as ctx:
        cpool = ctx.enter_context(tc.tile_pool(name="cpool", bufs=1))
        opool = ctx.enter_context(tc.tile_pool(name="opool", bufs=3))
        tpool = ctx.enter_context(tc.tile_pool(name="tpool", bufs=8))
        ppool = ctx.enter_context(tc.tile_pool(name="ppool", bufs=8,
                                               space="PSUM"))

        x_t = [cpool.tile([P, 2, BS], FP8, tag=f"x{t}", name=f"x_t{t}")
               for t in range(TP)]
        w_t = cpool.tile([P, IB, TP, 2, P], FP8, tag="w", name="w_t")
        xf_t = [cpool.tile([P, 2, BS], F16, tag=f"xf{t}", name=f"xf_t{t}")
                for t in range(TP)]
        vb_t = cpool.tile([P, 2 * IB], mybir.dt.float32, tag="vb",
                          name="vb_t")
        dv_t = vb_t[:, :IB]
        bv_t = vb_t[:, IB:]

        # DMA model (measured): a single transfer with >=4KB-per-partition
        # lines reaches near-full bandwidth (~512KB in ~1.5-3us); small or
        # thin-line pieces pay large fixed costs, each ring allows only ~4
        # in-flight transfers, and each trigger costs ~0.6-0.8us of
        # sequencer time. So: few, fat transfers, issued in the order the
        # data is consumed, balanced across the sync/scalar rings (loads)
        # with stores on the otherwise-idle gpsimd ring.
        #
        # gpsimd ring: the tiny thin-line diag/bias vector (it would poison
        # the head of a load ring), then stores later.
        nc.gpsimd.dma_start(out=vb_t, in_=vb)
        # sync ring: x8 k-pairs 0-2 (512KB, 4KB/part each), then fp16 x for
        # k-pairs 2-3 per (t,sub) = one eviction block each (512KB).
        # x8 k-pair 3 rides the otherwise-idle gpsimd ring so it lands
        # before the accumulation stream reaches it.
        for t in range(TP - 1):
            nc.sync.dma_start(out=x_t[t], in_=x8[:, t])
        nc.gpsimd.dma_start(out=x_t[TP - 1], in_=x8[:, TP - 1])
        for t in (2, 3):
            for sub in range(2):
                nc.sync.dma_start(out=xf_t[t][:, sub], in_=xf[:, t, sub])
        # scalar ring: w8 per ib-pair (256KB, 2KB/part), then fp16 x for
        # k-pairs 0-1.
        for i2 in range(IB // 2):
            nc.scalar.dma_start(out=w_t[:, bass.ts(i2, 2)],
                                in_=w8[:, bass.ts(i2, 2)])
        for t in (0, 1):
            for sub in range(2):
                nc.scalar.dma_start(out=xf_t[t][:, sub], in_=xf[:, t, sub])

        # PE pre-warm with fp8 DoubleRow matmuls (tile zeroed on the idle
        # gpsimd engine so warmup starts as soon as the PE boots).
        warm = cpool.tile([P, 2, 512], FP8, tag="warm", name="warm")
        nc.gpsimd.memset(warm, 0.0)
        warm_ps = ppool.tile([P, 512], mybir.dt.float32, tag="ps",
                             name="warm_ps")
        NWARM = 8
        for i in range(NWARM):
            nc.tensor.matmul(warm_ps, lhsT=warm[:, :, :P], rhs=warm,
                             start=(i == 0), stop=(i == NWARM - 1),
                             perf_mode=DR)

        for ib in range(IB):
            psums = [ppool.tile([P, 512], mybir.dt.float32, tag="ps",
                                name=f"ps{ib}_{bc}") for bc in range(NBC)]
            for t in range(TP):
                for bc in range(NBC):
                    nc.tensor.matmul(
                        psums[bc],
                        lhsT=w_t[:, ib, t],
                        rhs=x_t[t][:, :, bass.ts(bc, 512)],
                        start=(t == 0),
                        stop=(t == TP - 1),
                        perf_mode=DR,
                    )
            # eviction: out^T[i,b] = psum + d[i]*x_f16[i,b] + bias[i]
            t8, sub = divmod(ib, 2)
            o_t = opool.tile([P, BS], F16, tag="o", name=f"o_t{ib}")
            for bc in range(NBC):
                tb = tpool.tile([P, 512], F16, tag="tb",
                                name=f"tb{ib}_{bc}")
                nc.scalar.activation(tb, psums[bc], Act.Identity,
                                     bias=bv_t[:, ib:ib + 1], scale=1.0)
                nc.vector.scalar_tensor_tensor(
                    o_t[:, bass.ts(bc, 512)],
                    in0=xf_t[t8][:, sub, bass.ts(bc, 512)],
                    scalar=dv_t[:, ib:ib + 1],
                    in1=tb, op0=Alu.mult, op1=Alu.add)
                if ib == IB - 1:
                    # kernel tail: store per 512-chunk as soon as each
                    # chunk's eviction lands
                    nc.gpsimd.dma_start(out=out[:, ib, bass.ts(bc, 512)],
                                        in_=o_t[:, bass.ts(bc, 512)])
            if ib < IB - 1:
                nc.gpsimd.dma_start(out=out[:, ib], in_=o_t)

    nc.compile()
    return nc


def _get_nc():
    if "nc" not in _CACHE:
        _CACHE["nc"] = _build_nc()
    return _CACHE["nc"]


def _prep_inputs(x, lin_W, lin_b, W1, b1, W2):
    """Host-side prep: fold the basis chain, split W into off-diag + diag,
    quantize to fp8/fp16, and lay out transposed per core."""
    x = np.asarray(x, dtype=np.float32)
    lin_W = np.asarray(lin_W, dtype=np.float32)
    a = np.sum(np.asarray(W1, np.float32) * np.asarray(W2, np.float32),
               axis=1)
    c = np.sum(np.asarray(b1, np.float32) * np.asarray(W2, np.float32),
               axis=1)
    W_eff = np.ascontiguousarray(lin_W.T)
    idx = np.arange(D)
    W_eff[idx, idx] += a
    d = W_eff[idx, idx].copy()
    W_off = W_eff
    W_off[idx, idx] = 0.0
    bias = (np.asarray(lin_b, np.float32) + c).astype(np.float32)

    x8 = x.astype(FP8_NP)
    xf = x.astype(np.float16)
    w8 = W_off.astype(FP8_NP)

    # w8 dram layout [p, ib, t, 2, i128]: j = (2*t+sub)*128 + p,
    # i = ib*128 + i128
    w8_dev = np.ascontiguousarray(
        w8.reshape(TP, 2, P, IB, P).transpose(2, 3, 0, 1, 4))
    vb_dev = np.ascontiguousarray(
        np.concatenate([d.reshape(IB, P).T, bias.reshape(IB, P).T], axis=1))

    def xpose(arr):  # [NCORES*BS, D] -> per-core [p, t, 2, b]
        t = arr.reshape(NCORES, BS, TP, 2, P)
        return np.ascontiguousarray(t.transpose(0, 4, 2, 3, 1))

    x8_dev = xpose(x8)
    xf_dev = xpose(xf)

    return [
        {"x8": x8_dev[i], "xf": xf_dev[i], "w8": w8_dev, "vb": vb_dev}
        for i in range(NCORES)
    ]


def kernel(x, lin_W, lin_b, W1, b1, W2):
    in_maps = _prep_inputs(x, lin_W, lin_b, W1, b1, W2)
    nc = _get_nc()
    res = run_bass_kernel_spmd(nc, in_maps, core_ids=list(range(NCORES)))
    # out^T [p, ib, b] per core -> [b_global, ib*128+p]
    o = np.stack([r["out"] for r in res.results])  # [cores, P, IB, BS] fp16
    o = o.astype(np.float32).transpose(0, 3, 2, 1).reshape(B, D)
    return np.ascontiguousarray(o)
